# revision 21
# baseline (speedup 1.0000x reference)
"""CT projector (nn_CTProjector) on 8 Trainium2 NeuronCores.

Algorithm
---------
Rays = sources x dests pairs. For the reference geometry every ray's
y(t) depends only on (src_y, dst_y) ("a-line") and z(t) only on
(src_z, dst_z) ("b-line"), and all rays sharing the common tnear/tfar
("unclipped") sample the volume at identical parameters t_i.  At step i
all unclipped rays therefore lie in the same x-slab pair (x0, x0+1) with
a common fractional weight fx, and the trilinear sample for every
(a, b) ray simultaneously is the separable bilinear form

    val_i[a, b] = Uy(i)^T ( (1-fx) V[x0] + fx V[x0+1] ) Uz(i)

with 2-nonzero one-hot-interpolation matrices Uy [256y, A], Uz [256z, B].
The full sinogram integral is sum_i val_i — pure TensorEngine work
(two-stage matmul per step with PSUM accumulation), no gathers.

Sharding: the x-slab dimension (equivalently the step range) is split
across the 8 cores; each core holds only its ~34-slab window of the
volume (bf16, z-transposed) plus per-step one-hot matrices, accumulates
a partial [A, B] sinogram, and the host sums the partials.

Rays whose tfar differs from the common value (~8%, clipped by y/z box
faces) have a different step schedule; they are recomputed exactly on
the host (vectorized numpy) and overwrite the fast-path entries.

All per-core differences (slab windows, step schedules, 3-step-slab
irregularities) are encoded purely in the input data via a per-core
"virtual slab window": the single SPMD program iterates NITER virtual
slab pairs (W[v], W[v+1]); the host chooses each core's slot sequence W
and zero-pads unused step columns.
"""

import json

import numpy as np
import ml_dtypes

BF16 = ml_dtypes.bfloat16
FP8 = ml_dtypes.float8_e4m3
F32 = np.float32

N_CORES = 8

# ----------------------------------------------------------------------------
# Walrus in this container only accepts 1 sync-wait command per instruction.
# Split surplus waits onto injected same-engine NoOp carriers placed
# immediately before the original instruction (semaphores are monotonic
# within the kernel, so this is semantics-preserving).
# ----------------------------------------------------------------------------

_ENGINES_OK = {"PE", "DVE", "Activation", "Pool", "SP"}
_WAIT_LIMIT = 1


def _legalize_waits(bir_bytes):
    m = json.loads(bir_bytes)
    n_split = 0
    for fn in m.get("functions", []):
        for blk in fn.get("blocks", []):
            insts = blk.get("instructions")
            if not insts:
                continue
            out = []
            for ins in insts:
                si = ins.get("sync_info")
                ow = (si or {}).get("on_wait") or []
                eng = ins.get("engine")
                if len(ow) > _WAIT_LIMIT and eng in _ENGINES_OK:
                    surplus, keep = ow[:-_WAIT_LIMIT], ow[-_WAIT_LIMIT:]
                    for j, w in enumerate(surplus):
                        n_split += 1
                        out.append({
                            "debug": ins.get("debug", 0),
                            "engine": eng,
                            "ins": [],
                            "outs": [],
                            "name": f"{ins['name']}-wt{j}",
                            "opcode": "NoOp",
                            "sync_info": {"on_wait": [w], "on_update": []},
                        })
                    si["on_wait"] = keep
                out.append(ins)
            blk["instructions"] = out
    return json.dumps(m).encode(), n_split


_PATCHED = False


def _install_compile_patch():
    global _PATCHED
    if _PATCHED:
        return
    import concourse.bass_utils as bu
    import concourse.bass2jax as b2j
    orig = bu.compile_bir_kernel

    def patched(bir_json, tmpdir, neff_name="file.neff"):
        if isinstance(bir_json, str):
            bir_json = bir_json.encode()
        bir_json, _ = _legalize_waits(bir_json)
        return orig(bir_json, tmpdir, neff_name)

    bu.compile_bir_kernel = patched
    b2j.compile_bir_kernel = patched
    _PATCHED = True


# ----------------------------------------------------------------------------
# Host geometry (exact f32 replication of the reference arithmetic)
# ----------------------------------------------------------------------------

def _geometry(vols, sources, dests, vol_start, vol_spacing, num_steps):
    Ns, Nd = sources.shape[0], dests.shape[0]
    src = np.repeat(sources, Nd, axis=0).astype(F32)
    dst = np.tile(dests, (Ns, 1)).astype(F32)
    dims = np.array(vols.shape, dtype=F32)
    bmin = vol_start.astype(F32)
    bmax = (vol_start + vol_spacing * (dims - F32(1.0))).astype(F32)
    d = (dst - src).astype(F32)
    safe = np.where(np.abs(d) < 1e-9,
                    np.where(d < 0, F32(-1e-9), F32(1e-9)), d).astype(F32)
    inv = (F32(1.0) / safe).astype(F32)
    t0 = ((bmin - src) * inv).astype(F32)
    t1 = ((bmax - src) * inv).astype(F32)
    tnear = np.clip(np.max(np.minimum(t0, t1), axis=-1), 0.0, 1.0).astype(F32)
    tfar = np.clip(np.min(np.maximum(t0, t1), axis=-1), 0.0, 1.0).astype(F32)
    valid = tfar > tnear
    ray_len = np.sqrt(np.sum(d * d, axis=-1)).astype(F32)
    dt = (ray_len * (tfar - tnear) / F32(num_steps)).astype(F32)
    return dict(src=src, dst=dst, d=d, tnear=tnear, tfar=tfar, valid=valid,
                dt=dt, bmin=bmin, bmax=bmax, dims=dims, Ns=Ns, Nd=Nd)


def _raytrace_exact(vols, vol_start, vol_spacing, num_steps, g, rsel):
    """Exact numpy replication of the reference for a subset of rays."""
    src, dd = g['src'][rsel], g['d'][rsel]
    tnears, tfars = g['tnear'][rsel], g['tfar'][rsel]
    dts, valids = g['dt'][rsel], g['valid'][rsel]
    dims = g['dims']
    gmax = (dims - 1).astype(F32)
    i0max = (dims - 2).astype(np.int32)
    acc = np.zeros(rsel.sum(), dtype=F32)
    for i in range(num_steps):
        t = (tnears + (F32(i) + F32(0.5)) * (tfars - tnears)
             / F32(num_steps)).astype(F32)
        p = (src + t[:, None] * dd).astype(F32)
        gg = ((p - vol_start) / vol_spacing).astype(F32)
        i0 = np.clip(np.floor(gg).astype(np.int32), 0, i0max)
        f = np.clip(gg - i0.astype(F32), 0.0, 1.0).astype(F32)
        x0, y0, z0 = i0[:, 0], i0[:, 1], i0[:, 2]
        fx, fy, fz = f[:, 0], f[:, 1], f[:, 2]
        c000 = vols[x0, y0, z0]; c001 = vols[x0, y0, z0 + 1]
        c010 = vols[x0, y0 + 1, z0]; c011 = vols[x0, y0 + 1, z0 + 1]
        c100 = vols[x0 + 1, y0, z0]; c101 = vols[x0 + 1, y0, z0 + 1]
        c110 = vols[x0 + 1, y0 + 1, z0]; c111 = vols[x0 + 1, y0 + 1, z0 + 1]
        c00 = c000 * (1 - fz) + c001 * fz
        c01 = c010 * (1 - fz) + c011 * fz
        c10 = c100 * (1 - fz) + c101 * fz
        c11 = c110 * (1 - fz) + c111 * fz
        c0 = c00 * (1 - fy) + c01 * fy
        c1 = c10 * (1 - fy) + c11 * fy
        val = (c0 * (1 - fx) + c1 * fx).astype(F32)
        inb = np.all((gg >= 0.0) & (gg <= gmax), axis=-1)
        acc = (acc + np.where(inb & valids, dts * val, F32(0.0))).astype(F32)
    return acc


# ----------------------------------------------------------------------------
# Schedule / per-core data construction
# ----------------------------------------------------------------------------

class _Schedule:
    pass


def _build_schedule(vols, vol_start, vol_spacing, num_steps, g):
    """Returns None if the geometry is not regular enough for the fast path."""
    s = _Schedule()
    src, d, tnear, tfar, valid = g['src'], g['d'], g['tnear'], g['tfar'], g['valid']
    D, H, W_ = vols.shape
    if H > 256 or W_ > 256 or (H % 128) or (W_ % 128):
        return None

    if np.unique(tnear[valid]).size != 1:
        return None
    # x profile must be unique
    xk = np.unique(np.stack([src[:, 0], d[:, 0]], axis=1), axis=0)
    if xk.shape[0] != 1:
        return None
    tn = tnear[valid][0]
    tfc = np.max(tfar)
    unclipped = (tfar == tfc) & valid
    if not unclipped.any():
        return None

    # a / b line decomposition
    ya = np.stack([src[:, 1], d[:, 1]], axis=1)
    zb = np.stack([src[:, 2], d[:, 2]], axis=1)
    uy_keys, a_idx = np.unique(ya, axis=0, return_inverse=True)
    uz_keys, b_idx = np.unique(zb, axis=0, return_inverse=True)
    A, B = uy_keys.shape[0], uz_keys.shape[0]
    if A > 128 or B > 128:
        return None

    S = num_steps
    i_arr = np.arange(S, dtype=F32)
    t_i = (tn + (i_arr + F32(0.5)) * (tfc - tn) / F32(S)).astype(F32)

    # common x schedule
    px = (F32(xk[0, 0]) + t_i * F32(xk[0, 1])).astype(F32)
    gx = ((px - vol_start[0]) / vol_spacing[0]).astype(F32)
    x0 = np.clip(np.floor(gx).astype(np.int32), 0, D - 2)
    fxv = np.clip(gx - x0.astype(F32), 0.0, 1.0).astype(F32)
    inb_x = (gx >= 0) & (gx <= D - 1)

    dstep = np.diff(x0.astype(np.int64))
    if not (np.all(np.isin(dstep, (0, -1))) or np.all(np.isin(dstep, (0, 1)))):
        return None

    # y lines  [S, A]
    py = (uy_keys[None, :, 0].astype(F32)
          + t_i[:, None] * uy_keys[None, :, 1].astype(F32)).astype(F32)
    gy = ((py - vol_start[1]) / vol_spacing[1]).astype(F32)
    y0 = np.clip(np.floor(gy).astype(np.int32), 0, H - 2)
    wy1 = np.clip(gy - y0.astype(F32), 0.0, 1.0).astype(F32)
    wy0 = (F32(1.0) - wy1).astype(F32)
    inb_y = (gy >= 0) & (gy <= H - 1)
    # z lines  [S, B]
    pz = (uz_keys[None, :, 0].astype(F32)
          + t_i[:, None] * uz_keys[None, :, 1].astype(F32)).astype(F32)
    gz = ((pz - vol_start[2]) / vol_spacing[2]).astype(F32)
    z0 = np.clip(np.floor(gz).astype(np.int32), 0, W_ - 2)
    wz1 = np.clip(gz - z0.astype(F32), 0.0, 1.0).astype(F32)
    wz0 = (F32(1.0) - wz1).astype(F32)
    inb_z = (gz >= 0) & (gz <= W_ - 1)

    # per-core contiguous step ranges
    bounds = [int(round(S * k / N_CORES)) for k in range(N_CORES + 1)]

    cores = []
    for k in range(N_CORES):
        lo, hi = bounds[k], bounds[k + 1]
        steps = np.arange(lo, hi)
        # runs of equal x0
        runs = []
        for i in steps:
            sl = int(x0[i])
            if runs and runs[-1][0] == sl:
                runs[-1][1].append(i)
            else:
                runs.append([sl, [i]])
        # virtual window path W; pairs (W[v], W[v+1]); each pair <= 2 steps
        Wp = []
        pairs = []           # list of (slab, [step indices])
        for sl, idxs in runs:
            n = len(idxs)
            if n > 4:
                return None
            chunks = [idxs[j:j + 2] for j in range(0, n, 2)]
            for ch in chunks:
                lo_s, hi_s = sl, sl + 1
                if not Wp:
                    Wp.extend([lo_s, hi_s])
                    pairs.append((sl, ch))
                else:
                    cur = Wp[-1]
                    if cur == lo_s:
                        Wp.append(hi_s)
                        pairs.append((sl, ch))
                    elif cur == hi_s:
                        Wp.append(lo_s)
                        pairs.append((sl, ch))
                    else:
                        # transition pair with no steps, then the real one
                        Wp.append(lo_s if abs(cur - lo_s) <= abs(cur - hi_s)
                                  else hi_s)
                        pairs.append((-1, []))
                        Wp.append(hi_s if Wp[-1] == lo_s else lo_s)
                        pairs.append((sl, ch))
        cores.append((Wp, pairs))

    NITER = max(len(p) for _, p in cores)
    NSLOT = NITER + 1

    s.NITER, s.NSLOT = NITER, NSLOT
    s.A, s.B, s.S = A, B, S
    s.H, s.W_, s.D = H, W_, D
    s.a_idx, s.b_idx = a_idx, b_idx
    s.tn, s.tfc = tn, tfc
    s.unclipped = unclipped
    s.x0, s.fxv, s.inb_x = x0, fxv, inb_x
    s.y0, s.wy0, s.wy1, s.inb_y = y0, wy0, wy1, inb_y
    s.z0, s.wz0, s.wz1, s.inb_z = z0, wz0, wz1, inb_z
    s.cores = cores
    return s


def _build_core_inputs(vols, s):
    """Per-core numpy arrays: VW, UZ0, UZ1C, UY."""
    A, B = s.A, s.B
    NITER, NSLOT = s.NITER, s.NSLOT
    H, W_ = s.H, s.W_
    HC, WC = H // 128, W_ // 128  # chunks (2 for 256)
    in_maps = []
    vols_b = vols.astype(FP8)
    bcols = np.arange(B)
    acols = np.arange(A)
    for k in range(N_CORES):
        Wp, pairs = s.cores[k]
        Wp = list(Wp) + [Wp[-1]] * (NSLOT - len(Wp))
        pairs = list(pairs) + [(-1, [])] * (NITER - len(pairs))

        # VW: [NSLOT, 128(zp), WC(zc)*256(y)] fp8 ; lhsT chunk = [zp, 2zc, y]
        VW = np.zeros((NSLOT, 128, WC * H), dtype=FP8)
        for u in range(NSLOT):
            sl = min(max(Wp[u], 0), s.D - 1)
            slab = vols_b[sl]                       # [y, z]
            # [z, y] -> [zc, 128, y] -> [128, zc, y]
            zt = np.ascontiguousarray(slab.T).reshape(WC, 128, H)
            VW[u] = np.transpose(zt, (1, 0, 2)).reshape(128, WC * H)

        # UZ0/UZ1C: [NSLOT, 128(zp), WC(zc) * 2(col)*128(b)] f32 -> bf16
        UZ0 = np.zeros((NSLOT, 128, WC, 2 * 128), dtype=F32)
        UZ1 = np.zeros((NSLOT, 128, WC, 2 * 128), dtype=F32)
        # UY: [NITER, 128(yp), 2(col), HC(yc), 128(a)] f32 -> bf16
        UY = np.zeros((NITER, 128, 2, HC, 128), dtype=F32)

        for v, (sl, idxs) in enumerate(pairs):
            if sl < 0 or not idxs:
                continue
            w0s, w1s = Wp[v], Wp[v + 1]
            assert {w0s, w1s} == {sl, sl + 1}, (k, v, w0s, w1s, sl)
            for col, i in enumerate(idxs):
                if not s.inb_x[i]:
                    continue
                fx = s.fxv[i]
                xw_v = (F32(1.0) - fx) if w0s == sl else fx
                xw_n = fx if w0s == sl else (F32(1.0) - fx)
                mz = s.inb_z[i].astype(F32)
                zr = s.z0[i]
                cc = col * 128 + bcols
                UZ0[v, zr & 127, zr >> 7, cc] += xw_v * s.wz0[i] * mz
                UZ0[v, (zr + 1) & 127, (zr + 1) >> 7, cc] += xw_v * s.wz1[i] * mz
                UZ1[v + 1, zr & 127, zr >> 7, cc] += xw_n * s.wz0[i] * mz
                UZ1[v + 1, (zr + 1) & 127, (zr + 1) >> 7, cc] += xw_n * s.wz1[i] * mz
                my = s.inb_y[i].astype(F32)
                yr = s.y0[i]
                UY[v, yr & 127, col, yr >> 7, acols] += s.wy0[i] * my
                UY[v, (yr + 1) & 127, col, (yr + 1) >> 7, acols] += s.wy1[i] * my

        # pack per-iteration stream: [uz0 | vw slot v | uz1c | uy(pair v-2)]
        # (stage-1-critical uz0+vw land in the first half-slot DMA);
        # two consecutive slots interleaved per dram row for 4KB DMA lines
        NUST = NITER + 2
        UST = np.zeros((NUST, 128, 2048), dtype=FP8)
        UST[:NSLOT, :, 0:512] = UZ0.reshape(NSLOT, 128, 512).astype(FP8)
        UST[:NSLOT, :, 512:1024] = VW.reshape(NSLOT, 128, 512)
        UST[:NSLOT, :, 1024:1536] = UZ1.reshape(NSLOT, 128, 512).astype(FP8)
        uyb = UY.reshape(NITER, 128, 512).astype(FP8)
        UST[2:2 + NITER, :, 1536:2048] = uyb
        in_maps.append({"ust": UST})
    return in_maps


# ----------------------------------------------------------------------------
# Bass program
# ----------------------------------------------------------------------------

_NC_CACHE = {}


def _build_bass(NITER, NSLOT, HC, WC, H, W_):
    key = (NITER, NSLOT, HC, WC, H, W_)
    if key in _NC_CACHE:
        return _NC_CACHE[key]

    import concourse.bass as bass
    import concourse.mybir as mybir
    import concourse.tile as tile

    PSUM = bass.MemorySpace.PSUM
    nc = bass.Bass("TRN2", enable_partition_id=False)
    dt = mybir.dt

    NUST = NITER + 2
    ust_d = nc.dram_tensor("ust", [NUST, 128, 2048], dt.float8e4,
                           kind="ExternalInput")
    y_d = nc.dram_tensor("y_out", [128, 128], dt.bfloat16,
                         kind="ExternalOutput")
    DR = mybir.MatmulPerfMode.DoubleRow

    with tile.TileContext(nc) as tc:
        with (
            tc.tile_pool(name="ustp", bufs=12) as ustp,
            tc.tile_pool(name="sp", bufs=4) as spool,
            tc.tile_pool(name="outp", bufs=1) as outp,
            tc.tile_pool(name="wp", bufs=1) as wp,
            tc.tile_pool(name="bp", bufs=3, space=PSUM) as bp,
            tc.tile_pool(name="yp", bufs=1, space=PSUM) as yp,
            tc.tile_pool(name="wps", bufs=1, space=PSUM) as wps,
        ):
            y_ps = yp.tile([128, 128], dt.float32)

            # Stream DMAs alternate between the two hardware-DGE queues
            # (SP and Activation) so neither sequencer's ~600ns
            # descriptor-gen per transfer paces the stream.  Prefetch the
            # first two slots before the warm-up block.
            ust_tiles = {}

            def slot_dma(v, ut, split=False):
                eng = nc.sync if v % 2 == 0 else nc.scalar
                if split:
                    for q in range(2):
                        eng.dma_start(out=ut[:, q * 1024:(q + 1) * 1024],
                                      in_=ust_d[v, :, q * 1024:(q + 1) * 1024])
                else:
                    eng.dma_start(out=ut[:], in_=ust_d[v])

            for v in range(2):
                ut = ustp.tile([128, 2048], dt.float8e4, tag="ust")
                slot_dma(v, ut, split=True)
                ust_tiles[v] = ut

            # PE warm-up: dummy matmuls on zeroed scratch during the
            # prologue/first-DMA window so the HAM clock gate ramps
            # before real matmuls arrive.
            warm_in = wp.tile([128, 256], dt.bfloat16)
            nc.vector.memset(warm_in[:], 0.0)
            warm_ps = wps.tile([128, 256], dt.float32)
            for _w in range(6):
                nc.tensor.matmul(warm_ps[:, :], warm_in[:, 0:128],
                                 warm_in[:, :], start=True, stop=True)
            b_tiles = {}     # pair v -> [yt] psum tiles
            s_tiles = {}     # pair v -> sbuf fp8 tile
            n_s2 = 0
            first_s2 = True

            total_pairs = NITER
            s2_total = total_pairs * 2

            for v in range(NUST):
                if v in ust_tiles:
                    ut = ust_tiles[v]
                else:
                    ut = ustp.tile([128, 2048], dt.float8e4, tag="ust")
                    slot_dma(v, ut)
                    ust_tiles[v] = ut
                bt, off = ut, 0

                # fp8 DoubleRow: one matmul contracts both 128-z chunks.
                # lhsT [128z, 2zc, 128y@yt], rhs [128z, 2zc, 256cc]
                vw = bt[:, off + 512:off + 1024] \
                    .rearrange("p (zc y) -> p zc y", zc=WC)
                uz0 = bt[:, off + 0:off + 512] \
                    .rearrange("p (zc n) -> p zc n", zc=WC)
                uz1 = bt[:, off + 1024:off + 1536] \
                    .rearrange("p (zc n) -> p zc n", zc=WC)

                # ---- stage 1: side 0 (pair v) + side 1 (pair v-1),
                #      ordered so same-weights matmuls are adjacent ----
                s0 = v < total_pairs
                s1 = 1 <= v <= total_pairs
                if s0:
                    b_tiles[v] = [bp.tile([128, 256], dt.float32, tag=f"b{yt}",
                                          name=f"b{yt}_{v}")
                                  for yt in range(HC)]
                for yt in range(HC):
                    lt = vw[:, :, yt * 128:yt * 128 + 128]
                    if s0:
                        nc.tensor.matmul(
                            b_tiles[v][yt][:, :], lt, uz0,
                            start=True, stop=False, perf_mode=DR,
                        )
                    if s1:
                        nc.tensor.matmul(
                            b_tiles[v - 1][yt][:, :], lt, uz1,
                            start=False, stop=True, perf_mode=DR,
                        )
                if s1:
                    # copy completed B -> SBUF fp8 [128y, yc(2)*col(2)*b(128)]
                    pv = v - 1
                    st = spool.tile([128, 512], dt.float8e4, tag="s",
                                    name=f"s_{pv}")
                    for yt in range(HC):
                        dst = st[:, yt * 256:yt * 256 + 256]
                        if yt % 2 == 0:
                            nc.vector.tensor_copy(dst, b_tiles[pv][yt][:])
                        else:
                            nc.scalar.copy(dst, b_tiles[pv][yt][:])
                    s_tiles[pv] = st
                # ---- stage 2 (pair v-2), uy slices live in this iter's ut ----
                if v >= 2 and (v - 2) < total_pairs:
                    pv = v - 2
                    sv = s_tiles[pv][:].rearrange("p (yc n) -> p yc n", yc=HC)
                    for col in range(2):
                        n_s2 += 1
                        uy = bt[:, off + 1536 + col * 256:
                                off + 1536 + col * 256 + 256] \
                            .rearrange("p (yc a) -> p yc a", yc=HC)
                        nc.tensor.matmul(
                            y_ps[:, :],
                            uy,
                            sv[:, :, col * 128:col * 128 + 128],
                            start=first_s2, stop=(n_s2 == s2_total),
                            perf_mode=DR,
                        )
                        first_s2 = False
                    del s_tiles[pv]
                    del b_tiles[pv]

            y_sb = outp.tile([128, 128], dt.bfloat16)
            nc.vector.tensor_copy(y_sb[:], y_ps[:])
            # split across both HWDGE queues to halve output-DMA latency
            nc.sync.dma_start(out=y_d[:, 0:64], in_=y_sb[:, 0:64])
            nc.scalar.dma_start(out=y_d[:, 64:128], in_=y_sb[:, 64:128])

    _NC_CACHE[key] = nc
    return nc


# ----------------------------------------------------------------------------
# Entry point
# ----------------------------------------------------------------------------

def kernel(vols, sources, dests, vol_start, vol_spacing, num_steps):
    vols = np.asarray(vols, dtype=np.float32)
    sources = np.asarray(sources, dtype=np.float32)
    dests = np.asarray(dests, dtype=np.float32)
    vol_start = np.asarray(vol_start, dtype=np.float32)
    vol_spacing = np.asarray(vol_spacing, dtype=np.float32)
    num_steps = int(np.asarray(num_steps))

    g = _geometry(vols, sources, dests, vol_start, vol_spacing, num_steps)
    Ns, Nd = g['Ns'], g['Nd']
    R = Ns * Nd

    s = _build_schedule(vols, vol_start, vol_spacing, num_steps, g)
    if s is None:
        # irregular geometry: exact host fallback
        acc = _raytrace_exact(vols, vol_start, vol_spacing, num_steps, g,
                              np.ones(R, dtype=bool))
        out = np.where(g['valid'], acc, F32(0.0)).astype(F32)
        return out.reshape(Ns, Nd)

    in_maps = _build_core_inputs(vols, s)

    _install_compile_patch()
    from concourse.bass_utils import run_bass_kernel_spmd
    nc = _build_bass(s.NITER, s.NSLOT, s.H // 128, s.W_ // 128, s.H, s.W_)
    res = run_bass_kernel_spmd(nc, in_maps, core_ids=list(range(N_CORES)))
    global _LAST_RESULTS
    _LAST_RESULTS = res

    Y = np.zeros((128, 128), dtype=np.float64)
    for r in res.results:
        Y += r["y_out"].astype(np.float64)
    Y = Y.astype(F32)

    out = (g['dt'] * Y[s.a_idx, s.b_idx]).astype(F32)

    # exact recomputation for clipped / invalid rays
    fix = (~s.unclipped) & g['valid']
    if fix.any():
        out[fix] = _raytrace_exact(vols, vol_start, vol_spacing, num_steps,
                                   g, fix)
    out[~g['valid']] = 0.0
    return out.reshape(Ns, Nd)



# revision 22
# speedup vs baseline: 1.2571x; 1.2571x over previous
"""CT projector (nn_CTProjector) on 8 Trainium2 NeuronCores.

Algorithm
---------
Rays = sources x dests pairs. For the reference geometry every ray's
y(t) depends only on (src_y, dst_y) ("a-line") and z(t) only on
(src_z, dst_z) ("b-line"), and all rays sharing the common tnear/tfar
("unclipped") sample the volume at identical parameters t_i.  At step i
all unclipped rays therefore lie in the same x-slab pair (x0, x0+1) with
a common fractional weight fx, and the trilinear sample for every
(a, b) ray simultaneously is the separable bilinear form

    val_i[a, b] = Uy(i)^T ( (1-fx) V[x0] + fx V[x0+1] ) Uz(i)

with 2-nonzero one-hot-interpolation matrices Uy [256y, A], Uz [256z, B].
The full sinogram integral is sum_i val_i — pure TensorEngine work
(two-stage matmul per step with PSUM accumulation), no gathers.

Sharding: the x-slab dimension (equivalently the step range) is split
across the 8 cores; each core holds only its ~34-slab window of the
volume (bf16, z-transposed) plus per-step one-hot matrices, accumulates
a partial [A, B] sinogram, and the host sums the partials.

Rays whose tfar differs from the common value (~8%, clipped by y/z box
faces) have a different step schedule; they are recomputed exactly on
the host (vectorized numpy) and overwrite the fast-path entries.

All per-core differences (slab windows, step schedules, 3-step-slab
irregularities) are encoded purely in the input data via a per-core
"virtual slab window": the single SPMD program iterates NITER virtual
slab pairs (W[v], W[v+1]); the host chooses each core's slot sequence W
and zero-pads unused step columns.
"""

import json

import numpy as np
import ml_dtypes

BF16 = ml_dtypes.bfloat16
FP8 = ml_dtypes.float8_e4m3
F32 = np.float32

N_CORES = 8

# ----------------------------------------------------------------------------
# Walrus in this container only accepts 1 sync-wait command per instruction.
# Split surplus waits onto injected same-engine NoOp carriers placed
# immediately before the original instruction (semaphores are monotonic
# within the kernel, so this is semantics-preserving).
# ----------------------------------------------------------------------------

_ENGINES_OK = {"PE", "DVE", "Activation", "Pool", "SP"}
_WAIT_LIMIT = 1


def _legalize_waits(bir_bytes):
    m = json.loads(bir_bytes)
    n_split = 0
    for fn in m.get("functions", []):
        for blk in fn.get("blocks", []):
            insts = blk.get("instructions")
            if not insts:
                continue
            out = []
            for ins in insts:
                si = ins.get("sync_info")
                ow = (si or {}).get("on_wait") or []
                eng = ins.get("engine")
                if len(ow) > _WAIT_LIMIT and eng in _ENGINES_OK:
                    surplus, keep = ow[:-_WAIT_LIMIT], ow[-_WAIT_LIMIT:]
                    for j, w in enumerate(surplus):
                        n_split += 1
                        out.append({
                            "debug": ins.get("debug", 0),
                            "engine": eng,
                            "ins": [],
                            "outs": [],
                            "name": f"{ins['name']}-wt{j}",
                            "opcode": "NoOp",
                            "sync_info": {"on_wait": [w], "on_update": []},
                        })
                    si["on_wait"] = keep
                out.append(ins)
            blk["instructions"] = out
    return json.dumps(m).encode(), n_split


_PATCHED = False


def _install_compile_patch():
    global _PATCHED
    if _PATCHED:
        return
    import concourse.bass_utils as bu
    import concourse.bass2jax as b2j
    orig = bu.compile_bir_kernel

    def patched(bir_json, tmpdir, neff_name="file.neff"):
        if isinstance(bir_json, str):
            bir_json = bir_json.encode()
        bir_json, _ = _legalize_waits(bir_json)
        return orig(bir_json, tmpdir, neff_name)

    bu.compile_bir_kernel = patched
    b2j.compile_bir_kernel = patched
    _PATCHED = True


# ----------------------------------------------------------------------------
# Host geometry (exact f32 replication of the reference arithmetic)
# ----------------------------------------------------------------------------

def _geometry(vols, sources, dests, vol_start, vol_spacing, num_steps):
    Ns, Nd = sources.shape[0], dests.shape[0]
    src = np.repeat(sources, Nd, axis=0).astype(F32)
    dst = np.tile(dests, (Ns, 1)).astype(F32)
    dims = np.array(vols.shape, dtype=F32)
    bmin = vol_start.astype(F32)
    bmax = (vol_start + vol_spacing * (dims - F32(1.0))).astype(F32)
    d = (dst - src).astype(F32)
    safe = np.where(np.abs(d) < 1e-9,
                    np.where(d < 0, F32(-1e-9), F32(1e-9)), d).astype(F32)
    inv = (F32(1.0) / safe).astype(F32)
    t0 = ((bmin - src) * inv).astype(F32)
    t1 = ((bmax - src) * inv).astype(F32)
    tnear = np.clip(np.max(np.minimum(t0, t1), axis=-1), 0.0, 1.0).astype(F32)
    tfar = np.clip(np.min(np.maximum(t0, t1), axis=-1), 0.0, 1.0).astype(F32)
    valid = tfar > tnear
    ray_len = np.sqrt(np.sum(d * d, axis=-1)).astype(F32)
    dt = (ray_len * (tfar - tnear) / F32(num_steps)).astype(F32)
    return dict(src=src, dst=dst, d=d, tnear=tnear, tfar=tfar, valid=valid,
                dt=dt, bmin=bmin, bmax=bmax, dims=dims, Ns=Ns, Nd=Nd)


def _raytrace_exact(vols, vol_start, vol_spacing, num_steps, g, rsel):
    """Exact numpy replication of the reference for a subset of rays."""
    src, dd = g['src'][rsel], g['d'][rsel]
    tnears, tfars = g['tnear'][rsel], g['tfar'][rsel]
    dts, valids = g['dt'][rsel], g['valid'][rsel]
    dims = g['dims']
    gmax = (dims - 1).astype(F32)
    i0max = (dims - 2).astype(np.int32)
    acc = np.zeros(rsel.sum(), dtype=F32)
    for i in range(num_steps):
        t = (tnears + (F32(i) + F32(0.5)) * (tfars - tnears)
             / F32(num_steps)).astype(F32)
        p = (src + t[:, None] * dd).astype(F32)
        gg = ((p - vol_start) / vol_spacing).astype(F32)
        i0 = np.clip(np.floor(gg).astype(np.int32), 0, i0max)
        f = np.clip(gg - i0.astype(F32), 0.0, 1.0).astype(F32)
        x0, y0, z0 = i0[:, 0], i0[:, 1], i0[:, 2]
        fx, fy, fz = f[:, 0], f[:, 1], f[:, 2]
        c000 = vols[x0, y0, z0]; c001 = vols[x0, y0, z0 + 1]
        c010 = vols[x0, y0 + 1, z0]; c011 = vols[x0, y0 + 1, z0 + 1]
        c100 = vols[x0 + 1, y0, z0]; c101 = vols[x0 + 1, y0, z0 + 1]
        c110 = vols[x0 + 1, y0 + 1, z0]; c111 = vols[x0 + 1, y0 + 1, z0 + 1]
        c00 = c000 * (1 - fz) + c001 * fz
        c01 = c010 * (1 - fz) + c011 * fz
        c10 = c100 * (1 - fz) + c101 * fz
        c11 = c110 * (1 - fz) + c111 * fz
        c0 = c00 * (1 - fy) + c01 * fy
        c1 = c10 * (1 - fy) + c11 * fy
        val = (c0 * (1 - fx) + c1 * fx).astype(F32)
        inb = np.all((gg >= 0.0) & (gg <= gmax), axis=-1)
        acc = (acc + np.where(inb & valids, dts * val, F32(0.0))).astype(F32)
    return acc


# ----------------------------------------------------------------------------
# Schedule / per-core data construction
# ----------------------------------------------------------------------------

class _Schedule:
    pass


def _build_schedule(vols, vol_start, vol_spacing, num_steps, g):
    """Returns None if the geometry is not regular enough for the fast path."""
    s = _Schedule()
    src, d, tnear, tfar, valid = g['src'], g['d'], g['tnear'], g['tfar'], g['valid']
    D, H, W_ = vols.shape
    if H > 256 or W_ > 256 or (H % 128) or (W_ % 128):
        return None

    if np.unique(tnear[valid]).size != 1:
        return None
    # x profile must be unique
    xk = np.unique(np.stack([src[:, 0], d[:, 0]], axis=1), axis=0)
    if xk.shape[0] != 1:
        return None
    tn = tnear[valid][0]
    tfc = np.max(tfar)
    unclipped = (tfar == tfc) & valid
    if not unclipped.any():
        return None

    # a / b line decomposition
    ya = np.stack([src[:, 1], d[:, 1]], axis=1)
    zb = np.stack([src[:, 2], d[:, 2]], axis=1)
    uy_keys, a_idx = np.unique(ya, axis=0, return_inverse=True)
    uz_keys, b_idx = np.unique(zb, axis=0, return_inverse=True)
    A, B = uy_keys.shape[0], uz_keys.shape[0]
    if A > 128 or B > 128:
        return None

    S = num_steps
    i_arr = np.arange(S, dtype=F32)
    t_i = (tn + (i_arr + F32(0.5)) * (tfc - tn) / F32(S)).astype(F32)

    # common x schedule
    px = (F32(xk[0, 0]) + t_i * F32(xk[0, 1])).astype(F32)
    gx = ((px - vol_start[0]) / vol_spacing[0]).astype(F32)
    x0 = np.clip(np.floor(gx).astype(np.int32), 0, D - 2)
    fxv = np.clip(gx - x0.astype(F32), 0.0, 1.0).astype(F32)
    inb_x = (gx >= 0) & (gx <= D - 1)

    dstep = np.diff(x0.astype(np.int64))
    if not (np.all(np.isin(dstep, (0, -1))) or np.all(np.isin(dstep, (0, 1)))):
        return None

    # y lines  [S, A]
    py = (uy_keys[None, :, 0].astype(F32)
          + t_i[:, None] * uy_keys[None, :, 1].astype(F32)).astype(F32)
    gy = ((py - vol_start[1]) / vol_spacing[1]).astype(F32)
    y0 = np.clip(np.floor(gy).astype(np.int32), 0, H - 2)
    wy1 = np.clip(gy - y0.astype(F32), 0.0, 1.0).astype(F32)
    wy0 = (F32(1.0) - wy1).astype(F32)
    inb_y = (gy >= 0) & (gy <= H - 1)
    # z lines  [S, B]
    pz = (uz_keys[None, :, 0].astype(F32)
          + t_i[:, None] * uz_keys[None, :, 1].astype(F32)).astype(F32)
    gz = ((pz - vol_start[2]) / vol_spacing[2]).astype(F32)
    z0 = np.clip(np.floor(gz).astype(np.int32), 0, W_ - 2)
    wz1 = np.clip(gz - z0.astype(F32), 0.0, 1.0).astype(F32)
    wz0 = (F32(1.0) - wz1).astype(F32)
    inb_z = (gz >= 0) & (gz <= W_ - 1)

    # per-core contiguous step ranges
    bounds = [int(round(S * k / N_CORES)) for k in range(N_CORES + 1)]

    cores = []
    for k in range(N_CORES):
        lo, hi = bounds[k], bounds[k + 1]
        steps = np.arange(lo, hi)
        # runs of equal x0
        runs = []
        for i in steps:
            sl = int(x0[i])
            if runs and runs[-1][0] == sl:
                runs[-1][1].append(i)
            else:
                runs.append([sl, [i]])
        # virtual window path W; pairs (W[v], W[v+1]); each pair <= 2 steps
        Wp = []
        pairs = []           # list of (slab, [step indices])
        for sl, idxs in runs:
            n = len(idxs)
            if n > 4:
                return None
            chunks = [idxs[j:j + 2] for j in range(0, n, 2)]
            for ch in chunks:
                lo_s, hi_s = sl, sl + 1
                if not Wp:
                    Wp.extend([lo_s, hi_s])
                    pairs.append((sl, ch))
                else:
                    cur = Wp[-1]
                    if cur == lo_s:
                        Wp.append(hi_s)
                        pairs.append((sl, ch))
                    elif cur == hi_s:
                        Wp.append(lo_s)
                        pairs.append((sl, ch))
                    else:
                        # transition pair with no steps, then the real one
                        Wp.append(lo_s if abs(cur - lo_s) <= abs(cur - hi_s)
                                  else hi_s)
                        pairs.append((-1, []))
                        Wp.append(hi_s if Wp[-1] == lo_s else lo_s)
                        pairs.append((sl, ch))
        cores.append((Wp, pairs))

    NITER = max(len(p) for _, p in cores)
    NSLOT = NITER + 1

    s.NITER, s.NSLOT = NITER, NSLOT
    s.A, s.B, s.S = A, B, S
    s.H, s.W_, s.D = H, W_, D
    s.a_idx, s.b_idx = a_idx, b_idx
    s.tn, s.tfc = tn, tfc
    s.unclipped = unclipped
    s.x0, s.fxv, s.inb_x = x0, fxv, inb_x
    s.y0, s.wy0, s.wy1, s.inb_y = y0, wy0, wy1, inb_y
    s.z0, s.wz0, s.wz1, s.inb_z = z0, wz0, wz1, inb_z
    s.cores = cores
    return s


def _build_core_inputs(vols, s):
    """Per-core numpy arrays: VW, UZ0, UZ1C, UY."""
    A, B = s.A, s.B
    NITER, NSLOT = s.NITER, s.NSLOT
    H, W_ = s.H, s.W_
    HC, WC = H // 128, W_ // 128  # chunks (2 for 256)
    in_maps = []
    vols_b = vols.astype(FP8)
    bcols = np.arange(B)
    acols = np.arange(A)
    for k in range(N_CORES):
        Wp, pairs = s.cores[k]
        Wp = list(Wp) + [Wp[-1]] * (NSLOT - len(Wp))
        pairs = list(pairs) + [(-1, [])] * (NITER - len(pairs))

        # VW: [NSLOT, 128(zp), WC(zc)*256(y)] fp8 ; lhsT chunk = [zp, 2zc, y]
        VW = np.zeros((NSLOT, 128, WC * H), dtype=FP8)
        for u in range(NSLOT):
            sl = min(max(Wp[u], 0), s.D - 1)
            slab = vols_b[sl]                       # [y, z]
            # [z, y] -> [zc, 128, y] -> [128, zc, y]
            zt = np.ascontiguousarray(slab.T).reshape(WC, 128, H)
            VW[u] = np.transpose(zt, (1, 0, 2)).reshape(128, WC * H)

        # UZ0/UZ1C: [NSLOT, 128(zp), WC(zc) * 2(col)*128(b)] f32 -> bf16
        UZ0 = np.zeros((NSLOT, 128, WC, 2 * 128), dtype=F32)
        UZ1 = np.zeros((NSLOT, 128, WC, 2 * 128), dtype=F32)
        # UY: [NITER, 128(yp), 2(col), HC(yc), 128(a)] f32 -> bf16
        UY = np.zeros((NITER, 128, 2, HC, 128), dtype=F32)

        for v, (sl, idxs) in enumerate(pairs):
            if sl < 0 or not idxs:
                continue
            w0s, w1s = Wp[v], Wp[v + 1]
            assert {w0s, w1s} == {sl, sl + 1}, (k, v, w0s, w1s, sl)
            for col, i in enumerate(idxs):
                if not s.inb_x[i]:
                    continue
                fx = s.fxv[i]
                xw_v = (F32(1.0) - fx) if w0s == sl else fx
                xw_n = fx if w0s == sl else (F32(1.0) - fx)
                mz = s.inb_z[i].astype(F32)
                zr = s.z0[i]
                cc = col * 128 + bcols
                UZ0[v, zr & 127, zr >> 7, cc] += xw_v * s.wz0[i] * mz
                UZ0[v, (zr + 1) & 127, (zr + 1) >> 7, cc] += xw_v * s.wz1[i] * mz
                UZ1[v + 1, zr & 127, zr >> 7, cc] += xw_n * s.wz0[i] * mz
                UZ1[v + 1, (zr + 1) & 127, (zr + 1) >> 7, cc] += xw_n * s.wz1[i] * mz
                my = s.inb_y[i].astype(F32)
                yr = s.y0[i]
                UY[v, yr & 127, col, yr >> 7, acols] += s.wy0[i] * my
                UY[v, (yr + 1) & 127, col, (yr + 1) >> 7, acols] += s.wy1[i] * my

        # pack per-iteration stream: [uz0 | vw slot v | uz1c | uy(pair v-2)]
        # (stage-1-critical uz0+vw land in the first half-slot DMA);
        # two consecutive slots interleaved per dram row for 4KB DMA lines
        NUST = NITER + 2
        UST = np.zeros((NUST, 128, 2048), dtype=FP8)
        UST[:NSLOT, :, 0:512] = UZ0.reshape(NSLOT, 128, 512).astype(FP8)
        UST[:NSLOT, :, 512:1024] = VW.reshape(NSLOT, 128, 512)
        UST[:NSLOT, :, 1024:1536] = UZ1.reshape(NSLOT, 128, 512).astype(FP8)
        uyb = UY.reshape(NITER, 128, 512).astype(FP8)
        UST[2:2 + NITER, :, 1536:2048] = uyb
        in_maps.append({"ust": UST})
    return in_maps


# ----------------------------------------------------------------------------
# Bass program
# ----------------------------------------------------------------------------

_NC_CACHE = {}


def _build_bass(NITER, NSLOT, HC, WC, H, W_):
    key = (NITER, NSLOT, HC, WC, H, W_)
    if key in _NC_CACHE:
        return _NC_CACHE[key]

    import concourse.bass as bass
    import concourse.mybir as mybir
    import concourse.tile as tile

    PSUM = bass.MemorySpace.PSUM
    nc = bass.Bass("TRN2", enable_partition_id=False)
    dt = mybir.dt

    NUST = NITER + 2
    ust_d = nc.dram_tensor("ust", [NUST, 128, 2048], dt.float8e4,
                           kind="ExternalInput")
    y_d = nc.dram_tensor("y_out", [128, 128], dt.bfloat16,
                         kind="ExternalOutput")
    DR = mybir.MatmulPerfMode.DoubleRow

    with tile.TileContext(nc) as tc:
        with (
            tc.tile_pool(name="ustp", bufs=12) as ustp,
            tc.tile_pool(name="sp", bufs=4) as spool,
            tc.tile_pool(name="outp", bufs=1) as outp,
            tc.tile_pool(name="wp", bufs=1) as wp,
            tc.tile_pool(name="bp", bufs=3, space=PSUM) as bp,
            tc.tile_pool(name="yp", bufs=1, space=PSUM) as yp,
            tc.tile_pool(name="wps", bufs=1, space=PSUM) as wps,
        ):
            y_ps = yp.tile([128, 128], dt.float32)

            # Stream DMAs alternate between the two hardware-DGE queues
            # (SP and Activation) so neither sequencer's ~600ns
            # descriptor-gen per transfer paces the stream.  Prefetch the
            # first two slots before the warm-up block.
            ust_tiles = {}

            def slot_dma(v, ut, split=False):
                eng = nc.sync if v % 2 == 0 else nc.gpsimd
                if split:
                    for q in range(2):
                        eng.dma_start(out=ut[:, q * 1024:(q + 1) * 1024],
                                      in_=ust_d[v, :, q * 1024:(q + 1) * 1024])
                else:
                    eng.dma_start(out=ut[:], in_=ust_d[v])

            for v in range(2):
                ut = ustp.tile([128, 2048], dt.float8e4, tag="ust")
                slot_dma(v, ut, split=True)
                ust_tiles[v] = ut

            # PE warm-up: dummy matmuls on zeroed scratch during the
            # prologue/first-DMA window so the HAM clock gate ramps
            # before real matmuls arrive.
            warm_in = wp.tile([128, 256], dt.bfloat16)
            nc.vector.memset(warm_in[:], 0.0)
            warm_ps = wps.tile([128, 256], dt.float32)
            for _w in range(6):
                nc.tensor.matmul(warm_ps[:, :], warm_in[:, 0:128],
                                 warm_in[:, :], start=True, stop=True)
            b_tiles = {}     # pair v -> [yt] psum tiles
            s_tiles = {}     # pair v -> sbuf fp8 tile
            n_s2 = 0
            first_s2 = True

            total_pairs = NITER
            s2_total = total_pairs * 2

            for v in range(NUST):
                if v in ust_tiles:
                    ut = ust_tiles[v]
                else:
                    ut = ustp.tile([128, 2048], dt.float8e4, tag="ust")
                    slot_dma(v, ut)
                    ust_tiles[v] = ut
                bt, off = ut, 0

                # fp8 DoubleRow: one matmul contracts both 128-z chunks.
                # lhsT [128z, 2zc, 128y@yt], rhs [128z, 2zc, 256cc]
                vw = bt[:, off + 512:off + 1024] \
                    .rearrange("p (zc y) -> p zc y", zc=WC)
                uz0 = bt[:, off + 0:off + 512] \
                    .rearrange("p (zc n) -> p zc n", zc=WC)
                uz1 = bt[:, off + 1024:off + 1536] \
                    .rearrange("p (zc n) -> p zc n", zc=WC)

                # ---- stage 1: side 0 (pair v) + side 1 (pair v-1),
                #      ordered so same-weights matmuls are adjacent ----
                s0 = v < total_pairs
                s1 = 1 <= v <= total_pairs
                if s0:
                    b_tiles[v] = [bp.tile([128, 256], dt.float32, tag=f"b{yt}",
                                          name=f"b{yt}_{v}")
                                  for yt in range(HC)]
                for yt in range(HC):
                    lt = vw[:, :, yt * 128:yt * 128 + 128]
                    if s0:
                        nc.tensor.matmul(
                            b_tiles[v][yt][:, :], lt, uz0,
                            start=True, stop=False, perf_mode=DR,
                        )
                    if s1:
                        nc.tensor.matmul(
                            b_tiles[v - 1][yt][:, :], lt, uz1,
                            start=False, stop=True, perf_mode=DR,
                        )
                if s1:
                    # copy completed B -> SBUF fp8 [128y, yc(2)*col(2)*b(128)]
                    pv = v - 1
                    st = spool.tile([128, 512], dt.float8e4, tag="s",
                                    name=f"s_{pv}")
                    for yt in range(HC):
                        dst = st[:, yt * 256:yt * 256 + 256]
                        if yt % 2 == 0:
                            nc.vector.tensor_copy(dst, b_tiles[pv][yt][:])
                        else:
                            nc.scalar.copy(dst, b_tiles[pv][yt][:])
                    s_tiles[pv] = st
                # ---- stage 2 (pair v-2), uy slices live in this iter's ut ----
                if v >= 2 and (v - 2) < total_pairs:
                    pv = v - 2
                    sv = s_tiles[pv][:].rearrange("p (yc n) -> p yc n", yc=HC)
                    for col in range(2):
                        n_s2 += 1
                        uy = bt[:, off + 1536 + col * 256:
                                off + 1536 + col * 256 + 256] \
                            .rearrange("p (yc a) -> p yc a", yc=HC)
                        nc.tensor.matmul(
                            y_ps[:, :],
                            uy,
                            sv[:, :, col * 128:col * 128 + 128],
                            start=first_s2, stop=(n_s2 == s2_total),
                            perf_mode=DR,
                        )
                        first_s2 = False
                    del s_tiles[pv]
                    del b_tiles[pv]

            y_sb = outp.tile([128, 128], dt.bfloat16)
            nc.vector.tensor_copy(y_sb[:], y_ps[:])
            # split across both HWDGE queues to halve output-DMA latency
            nc.sync.dma_start(out=y_d[:, 0:64], in_=y_sb[:, 0:64])
            nc.scalar.dma_start(out=y_d[:, 64:128], in_=y_sb[:, 64:128])

    _NC_CACHE[key] = nc
    return nc


# ----------------------------------------------------------------------------
# Entry point
# ----------------------------------------------------------------------------

def kernel(vols, sources, dests, vol_start, vol_spacing, num_steps):
    vols = np.asarray(vols, dtype=np.float32)
    sources = np.asarray(sources, dtype=np.float32)
    dests = np.asarray(dests, dtype=np.float32)
    vol_start = np.asarray(vol_start, dtype=np.float32)
    vol_spacing = np.asarray(vol_spacing, dtype=np.float32)
    num_steps = int(np.asarray(num_steps))

    g = _geometry(vols, sources, dests, vol_start, vol_spacing, num_steps)
    Ns, Nd = g['Ns'], g['Nd']
    R = Ns * Nd

    s = _build_schedule(vols, vol_start, vol_spacing, num_steps, g)
    if s is None:
        # irregular geometry: exact host fallback
        acc = _raytrace_exact(vols, vol_start, vol_spacing, num_steps, g,
                              np.ones(R, dtype=bool))
        out = np.where(g['valid'], acc, F32(0.0)).astype(F32)
        return out.reshape(Ns, Nd)

    in_maps = _build_core_inputs(vols, s)

    _install_compile_patch()
    from concourse.bass_utils import run_bass_kernel_spmd
    nc = _build_bass(s.NITER, s.NSLOT, s.H // 128, s.W_ // 128, s.H, s.W_)
    res = run_bass_kernel_spmd(nc, in_maps, core_ids=list(range(N_CORES)))
    global _LAST_RESULTS
    _LAST_RESULTS = res

    Y = np.zeros((128, 128), dtype=np.float64)
    for r in res.results:
        Y += r["y_out"].astype(np.float64)
    Y = Y.astype(F32)

    out = (g['dt'] * Y[s.a_idx, s.b_idx]).astype(F32)

    # exact recomputation for clipped / invalid rays
    fix = (~s.unclipped) & g['valid']
    if fix.any():
        out[fix] = _raytrace_exact(vols, vol_start, vol_spacing, num_steps,
                                   g, fix)
    out[~g['valid']] = 0.0
    return out.reshape(Ns, Nd)



# revision 24
# speedup vs baseline: 1.5146x; 1.2048x over previous
"""CT projector (nn_CTProjector) on 8 Trainium2 NeuronCores.

Algorithm
---------
Rays = sources x dests pairs. For the reference geometry every ray's
y(t) depends only on (src_y, dst_y) ("a-line") and z(t) only on
(src_z, dst_z) ("b-line"), and all rays sharing the common tnear/tfar
("unclipped") sample the volume at identical parameters t_i.  At step i
all unclipped rays therefore lie in the same x-slab pair (x0, x0+1) with
a common fractional weight fx, and the trilinear sample for every
(a, b) ray simultaneously is the separable bilinear form

    val_i[a, b] = Uy(i)^T ( (1-fx) V[x0] + fx V[x0+1] ) Uz(i)

with 2-nonzero one-hot-interpolation matrices Uy [256y, A], Uz [256z, B].
The full sinogram integral is sum_i val_i — pure TensorEngine work
(two-stage matmul per step with PSUM accumulation), no gathers.

Sharding: the x-slab dimension (equivalently the step range) is split
across the 8 cores; each core holds only its ~34-slab window of the
volume (bf16, z-transposed) plus per-step one-hot matrices, accumulates
a partial [A, B] sinogram, and the host sums the partials.

Rays whose tfar differs from the common value (~8%, clipped by y/z box
faces) have a different step schedule; they are recomputed exactly on
the host (vectorized numpy) and overwrite the fast-path entries.

All per-core differences (slab windows, step schedules, 3-step-slab
irregularities) are encoded purely in the input data via a per-core
"virtual slab window": the single SPMD program iterates NITER virtual
slab pairs (W[v], W[v+1]); the host chooses each core's slot sequence W
and zero-pads unused step columns.
"""

import json

import numpy as np
import ml_dtypes

BF16 = ml_dtypes.bfloat16
FP8 = ml_dtypes.float8_e4m3
F32 = np.float32

N_CORES = 8

# ----------------------------------------------------------------------------
# Walrus in this container only accepts 1 sync-wait command per instruction.
# Split surplus waits onto injected same-engine NoOp carriers placed
# immediately before the original instruction (semaphores are monotonic
# within the kernel, so this is semantics-preserving).
# ----------------------------------------------------------------------------

_ENGINES_OK = {"PE", "DVE", "Activation", "Pool", "SP"}
_WAIT_LIMIT = 1


def _legalize_waits(bir_bytes):
    m = json.loads(bir_bytes)
    n_split = 0
    for fn in m.get("functions", []):
        for blk in fn.get("blocks", []):
            insts = blk.get("instructions")
            if not insts:
                continue
            out = []
            for ins in insts:
                si = ins.get("sync_info")
                ow = (si or {}).get("on_wait") or []
                eng = ins.get("engine")
                if len(ow) > _WAIT_LIMIT and eng in _ENGINES_OK:
                    surplus, keep = ow[:-_WAIT_LIMIT], ow[-_WAIT_LIMIT:]
                    for j, w in enumerate(surplus):
                        n_split += 1
                        out.append({
                            "debug": ins.get("debug", 0),
                            "engine": eng,
                            "ins": [],
                            "outs": [],
                            "name": f"{ins['name']}-wt{j}",
                            "opcode": "NoOp",
                            "sync_info": {"on_wait": [w], "on_update": []},
                        })
                    si["on_wait"] = keep
                out.append(ins)
            blk["instructions"] = out
    return json.dumps(m).encode(), n_split


_PATCHED = False


def _install_compile_patch():
    global _PATCHED
    if _PATCHED:
        return
    import concourse.bass_utils as bu
    import concourse.bass2jax as b2j
    orig = bu.compile_bir_kernel

    def patched(bir_json, tmpdir, neff_name="file.neff"):
        if isinstance(bir_json, str):
            bir_json = bir_json.encode()
        bir_json, _ = _legalize_waits(bir_json)
        return orig(bir_json, tmpdir, neff_name)

    bu.compile_bir_kernel = patched
    b2j.compile_bir_kernel = patched
    _PATCHED = True


# ----------------------------------------------------------------------------
# Host geometry (exact f32 replication of the reference arithmetic)
# ----------------------------------------------------------------------------

def _geometry(vols, sources, dests, vol_start, vol_spacing, num_steps):
    Ns, Nd = sources.shape[0], dests.shape[0]
    src = np.repeat(sources, Nd, axis=0).astype(F32)
    dst = np.tile(dests, (Ns, 1)).astype(F32)
    dims = np.array(vols.shape, dtype=F32)
    bmin = vol_start.astype(F32)
    bmax = (vol_start + vol_spacing * (dims - F32(1.0))).astype(F32)
    d = (dst - src).astype(F32)
    safe = np.where(np.abs(d) < 1e-9,
                    np.where(d < 0, F32(-1e-9), F32(1e-9)), d).astype(F32)
    inv = (F32(1.0) / safe).astype(F32)
    t0 = ((bmin - src) * inv).astype(F32)
    t1 = ((bmax - src) * inv).astype(F32)
    tnear = np.clip(np.max(np.minimum(t0, t1), axis=-1), 0.0, 1.0).astype(F32)
    tfar = np.clip(np.min(np.maximum(t0, t1), axis=-1), 0.0, 1.0).astype(F32)
    valid = tfar > tnear
    ray_len = np.sqrt(np.sum(d * d, axis=-1)).astype(F32)
    dt = (ray_len * (tfar - tnear) / F32(num_steps)).astype(F32)
    return dict(src=src, dst=dst, d=d, tnear=tnear, tfar=tfar, valid=valid,
                dt=dt, bmin=bmin, bmax=bmax, dims=dims, Ns=Ns, Nd=Nd)


def _raytrace_exact(vols, vol_start, vol_spacing, num_steps, g, rsel):
    """Exact numpy replication of the reference for a subset of rays."""
    src, dd = g['src'][rsel], g['d'][rsel]
    tnears, tfars = g['tnear'][rsel], g['tfar'][rsel]
    dts, valids = g['dt'][rsel], g['valid'][rsel]
    dims = g['dims']
    gmax = (dims - 1).astype(F32)
    i0max = (dims - 2).astype(np.int32)
    acc = np.zeros(rsel.sum(), dtype=F32)
    for i in range(num_steps):
        t = (tnears + (F32(i) + F32(0.5)) * (tfars - tnears)
             / F32(num_steps)).astype(F32)
        p = (src + t[:, None] * dd).astype(F32)
        gg = ((p - vol_start) / vol_spacing).astype(F32)
        i0 = np.clip(np.floor(gg).astype(np.int32), 0, i0max)
        f = np.clip(gg - i0.astype(F32), 0.0, 1.0).astype(F32)
        x0, y0, z0 = i0[:, 0], i0[:, 1], i0[:, 2]
        fx, fy, fz = f[:, 0], f[:, 1], f[:, 2]
        c000 = vols[x0, y0, z0]; c001 = vols[x0, y0, z0 + 1]
        c010 = vols[x0, y0 + 1, z0]; c011 = vols[x0, y0 + 1, z0 + 1]
        c100 = vols[x0 + 1, y0, z0]; c101 = vols[x0 + 1, y0, z0 + 1]
        c110 = vols[x0 + 1, y0 + 1, z0]; c111 = vols[x0 + 1, y0 + 1, z0 + 1]
        c00 = c000 * (1 - fz) + c001 * fz
        c01 = c010 * (1 - fz) + c011 * fz
        c10 = c100 * (1 - fz) + c101 * fz
        c11 = c110 * (1 - fz) + c111 * fz
        c0 = c00 * (1 - fy) + c01 * fy
        c1 = c10 * (1 - fy) + c11 * fy
        val = (c0 * (1 - fx) + c1 * fx).astype(F32)
        inb = np.all((gg >= 0.0) & (gg <= gmax), axis=-1)
        acc = (acc + np.where(inb & valids, dts * val, F32(0.0))).astype(F32)
    return acc


# ----------------------------------------------------------------------------
# Schedule / per-core data construction
# ----------------------------------------------------------------------------

class _Schedule:
    pass


def _build_schedule(vols, vol_start, vol_spacing, num_steps, g):
    """Returns None if the geometry is not regular enough for the fast path."""
    s = _Schedule()
    src, d, tnear, tfar, valid = g['src'], g['d'], g['tnear'], g['tfar'], g['valid']
    D, H, W_ = vols.shape
    if H > 256 or W_ > 256 or (H % 128) or (W_ % 128):
        return None

    if np.unique(tnear[valid]).size != 1:
        return None
    # x profile must be unique
    xk = np.unique(np.stack([src[:, 0], d[:, 0]], axis=1), axis=0)
    if xk.shape[0] != 1:
        return None
    tn = tnear[valid][0]
    tfc = np.max(tfar)
    unclipped = (tfar == tfc) & valid
    if not unclipped.any():
        return None

    # a / b line decomposition
    ya = np.stack([src[:, 1], d[:, 1]], axis=1)
    zb = np.stack([src[:, 2], d[:, 2]], axis=1)
    uy_keys, a_idx = np.unique(ya, axis=0, return_inverse=True)
    uz_keys, b_idx = np.unique(zb, axis=0, return_inverse=True)
    A, B = uy_keys.shape[0], uz_keys.shape[0]
    if A > 128 or B > 128:
        return None

    S = num_steps
    i_arr = np.arange(S, dtype=F32)
    t_i = (tn + (i_arr + F32(0.5)) * (tfc - tn) / F32(S)).astype(F32)

    # common x schedule
    px = (F32(xk[0, 0]) + t_i * F32(xk[0, 1])).astype(F32)
    gx = ((px - vol_start[0]) / vol_spacing[0]).astype(F32)
    x0 = np.clip(np.floor(gx).astype(np.int32), 0, D - 2)
    fxv = np.clip(gx - x0.astype(F32), 0.0, 1.0).astype(F32)
    inb_x = (gx >= 0) & (gx <= D - 1)

    dstep = np.diff(x0.astype(np.int64))
    if not (np.all(np.isin(dstep, (0, -1))) or np.all(np.isin(dstep, (0, 1)))):
        return None

    # y lines  [S, A]
    py = (uy_keys[None, :, 0].astype(F32)
          + t_i[:, None] * uy_keys[None, :, 1].astype(F32)).astype(F32)
    gy = ((py - vol_start[1]) / vol_spacing[1]).astype(F32)
    y0 = np.clip(np.floor(gy).astype(np.int32), 0, H - 2)
    wy1 = np.clip(gy - y0.astype(F32), 0.0, 1.0).astype(F32)
    wy0 = (F32(1.0) - wy1).astype(F32)
    inb_y = (gy >= 0) & (gy <= H - 1)
    # z lines  [S, B]
    pz = (uz_keys[None, :, 0].astype(F32)
          + t_i[:, None] * uz_keys[None, :, 1].astype(F32)).astype(F32)
    gz = ((pz - vol_start[2]) / vol_spacing[2]).astype(F32)
    z0 = np.clip(np.floor(gz).astype(np.int32), 0, W_ - 2)
    wz1 = np.clip(gz - z0.astype(F32), 0.0, 1.0).astype(F32)
    wz0 = (F32(1.0) - wz1).astype(F32)
    inb_z = (gz >= 0) & (gz <= W_ - 1)

    # per-core contiguous step ranges
    bounds = [int(round(S * k / N_CORES)) for k in range(N_CORES + 1)]

    cores = []
    for k in range(N_CORES):
        lo, hi = bounds[k], bounds[k + 1]
        steps = np.arange(lo, hi)
        # runs of equal x0
        runs = []
        for i in steps:
            sl = int(x0[i])
            if runs and runs[-1][0] == sl:
                runs[-1][1].append(i)
            else:
                runs.append([sl, [i]])
        # virtual window path W; pairs (W[v], W[v+1]); each pair <= 2 steps
        Wp = []
        pairs = []           # list of (slab, [step indices])
        for sl, idxs in runs:
            n = len(idxs)
            if n > 4:
                return None
            chunks = [idxs[j:j + 2] for j in range(0, n, 2)]
            for ch in chunks:
                lo_s, hi_s = sl, sl + 1
                if not Wp:
                    Wp.extend([lo_s, hi_s])
                    pairs.append((sl, ch))
                else:
                    cur = Wp[-1]
                    if cur == lo_s:
                        Wp.append(hi_s)
                        pairs.append((sl, ch))
                    elif cur == hi_s:
                        Wp.append(lo_s)
                        pairs.append((sl, ch))
                    else:
                        # transition pair with no steps, then the real one
                        Wp.append(lo_s if abs(cur - lo_s) <= abs(cur - hi_s)
                                  else hi_s)
                        pairs.append((-1, []))
                        Wp.append(hi_s if Wp[-1] == lo_s else lo_s)
                        pairs.append((sl, ch))
        cores.append((Wp, pairs))

    NITER = max(len(p) for _, p in cores)
    NSLOT = NITER + 1

    s.NITER, s.NSLOT = NITER, NSLOT
    s.A, s.B, s.S = A, B, S
    s.H, s.W_, s.D = H, W_, D
    s.a_idx, s.b_idx = a_idx, b_idx
    s.tn, s.tfc = tn, tfc
    s.unclipped = unclipped
    s.x0, s.fxv, s.inb_x = x0, fxv, inb_x
    s.y0, s.wy0, s.wy1, s.inb_y = y0, wy0, wy1, inb_y
    s.z0, s.wz0, s.wz1, s.inb_z = z0, wz0, wz1, inb_z
    s.cores = cores
    return s


def _build_core_inputs(vols, s):
    """Per-core numpy arrays: VW, UZ0, UZ1C, UY."""
    A, B = s.A, s.B
    NITER, NSLOT = s.NITER, s.NSLOT
    H, W_ = s.H, s.W_
    HC, WC = H // 128, W_ // 128  # chunks (2 for 256)
    in_maps = []
    vols_b = vols.astype(FP8)
    bcols = np.arange(B)
    acols = np.arange(A)
    for k in range(N_CORES):
        Wp, pairs = s.cores[k]
        Wp = list(Wp) + [Wp[-1]] * (NSLOT - len(Wp))
        pairs = list(pairs) + [(-1, [])] * (NITER - len(pairs))

        # VW: [NSLOT, 128(zp), WC(zc)*256(y)] fp8 ; lhsT chunk = [zp, 2zc, y]
        VW = np.zeros((NSLOT, 128, WC * H), dtype=FP8)
        for u in range(NSLOT):
            sl = min(max(Wp[u], 0), s.D - 1)
            slab = vols_b[sl]                       # [y, z]
            # [z, y] -> [zc, 128, y] -> [128, zc, y]
            zt = np.ascontiguousarray(slab.T).reshape(WC, 128, H)
            VW[u] = np.transpose(zt, (1, 0, 2)).reshape(128, WC * H)

        # UZ0/UZ1C: [NSLOT, 128(zp), WC(zc) * 2(col)*128(b)] f32 -> bf16
        UZ0 = np.zeros((NSLOT, 128, WC, 2 * 128), dtype=F32)
        UZ1 = np.zeros((NSLOT, 128, WC, 2 * 128), dtype=F32)
        # UY: [NITER, 128(yp), 2(col), HC(yc), 128(a)] f32 -> bf16
        UY = np.zeros((NITER, 128, 2, HC, 128), dtype=F32)

        for v, (sl, idxs) in enumerate(pairs):
            if sl < 0 or not idxs:
                continue
            w0s, w1s = Wp[v], Wp[v + 1]
            assert {w0s, w1s} == {sl, sl + 1}, (k, v, w0s, w1s, sl)
            for col, i in enumerate(idxs):
                if not s.inb_x[i]:
                    continue
                fx = s.fxv[i]
                xw_v = (F32(1.0) - fx) if w0s == sl else fx
                xw_n = fx if w0s == sl else (F32(1.0) - fx)
                mz = s.inb_z[i].astype(F32)
                zr = s.z0[i]
                cc = col * 128 + bcols
                UZ0[v, zr & 127, zr >> 7, cc] += xw_v * s.wz0[i] * mz
                UZ0[v, (zr + 1) & 127, (zr + 1) >> 7, cc] += xw_v * s.wz1[i] * mz
                UZ1[v + 1, zr & 127, zr >> 7, cc] += xw_n * s.wz0[i] * mz
                UZ1[v + 1, (zr + 1) & 127, (zr + 1) >> 7, cc] += xw_n * s.wz1[i] * mz
                my = s.inb_y[i].astype(F32)
                yr = s.y0[i]
                UY[v, yr & 127, col, yr >> 7, acols] += s.wy0[i] * my
                UY[v, (yr + 1) & 127, col, (yr + 1) >> 7, acols] += s.wy1[i] * my

        # pack per-iteration stream: [uz0 | vw slot v | uz1c | uy(pair v-2)]
        # (stage-1-critical uz0+vw land in the first half-slot DMA);
        # two consecutive slots interleaved per dram row for 4KB DMA lines
        NUST = NITER + 2
        UST = np.zeros((NUST, 128, 2048), dtype=FP8)
        UST[:NSLOT, :, 0:512] = UZ0.reshape(NSLOT, 128, 512).astype(FP8)
        UST[:NSLOT, :, 512:1024] = VW.reshape(NSLOT, 128, 512)
        UST[:NSLOT, :, 1024:1536] = UZ1.reshape(NSLOT, 128, 512).astype(FP8)
        uyb = UY.reshape(NITER, 128, 512).astype(FP8)
        UST[2:2 + NITER, :, 1536:2048] = uyb
        in_maps.append({"ust": UST})
    return in_maps


# ----------------------------------------------------------------------------
# Bass program
# ----------------------------------------------------------------------------

_NC_CACHE = {}


def _build_bass(NITER, NSLOT, HC, WC, H, W_):
    key = (NITER, NSLOT, HC, WC, H, W_)
    if key in _NC_CACHE:
        return _NC_CACHE[key]

    import concourse.bass as bass
    import concourse.mybir as mybir
    import concourse.tile as tile

    PSUM = bass.MemorySpace.PSUM
    nc = bass.Bass("TRN2", enable_partition_id=False)
    dt = mybir.dt

    NUST = NITER + 2
    ust_d = nc.dram_tensor("ust", [NUST, 128, 2048], dt.float8e4,
                           kind="ExternalInput")
    y_d = nc.dram_tensor("y_out", [128, 128], dt.bfloat16,
                         kind="ExternalOutput")
    DR = mybir.MatmulPerfMode.DoubleRow

    with tile.TileContext(nc) as tc:
        with (
            tc.tile_pool(name="ustp", bufs=12) as ustp,
            tc.tile_pool(name="sp", bufs=4) as spool,
            tc.tile_pool(name="outp", bufs=1) as outp,
            tc.tile_pool(name="wp", bufs=1) as wp,
            tc.tile_pool(name="bp", bufs=3, space=PSUM) as bp,
            tc.tile_pool(name="yp", bufs=1, space=PSUM) as yp,
            tc.tile_pool(name="wps", bufs=1, space=PSUM) as wps,
        ):
            y_ps = yp.tile([128, 128], dt.float32)

            # Stream DMAs alternate between the two hardware-DGE queues
            # (SP and Activation) so neither sequencer's ~600ns
            # descriptor-gen per transfer paces the stream.  Prefetch the
            # first two slots before the warm-up block.
            ust_tiles = {}

            NSLOT_ = NITER + 1  # slots with live uz0/vw/uz1c data

            def slot_dma(v, ut, split=False):
                # trim the zero-padded regions: slot 0 has no uz1c/uy,
                # slot 1 no uy, slots >= NSLOT_ only uy.
                if v == 0:
                    lo, hi = 0, 1024
                elif v == 1:
                    lo, hi = 0, 1536
                elif v >= NSLOT_:
                    lo, hi = 1536, 2048
                else:
                    lo, hi = 0, 2048
                if split and hi - lo > 1024:
                    mid = lo + 1024
                    nc.sync.dma_start(out=ut[:, lo:mid],
                                      in_=ust_d[v, :, lo:mid])
                    nc.sync.dma_start(out=ut[:, mid:hi],
                                      in_=ust_d[v, :, mid:hi])
                else:
                    nc.sync.dma_start(out=ut[:, lo:hi], in_=ust_d[v, :, lo:hi])

            for v in range(2):
                ut = ustp.tile([128, 2048], dt.float8e4, tag="ust")
                slot_dma(v, ut, split=True)
                ust_tiles[v] = ut

            # PE warm-up: dummy matmuls on zeroed scratch during the
            # prologue/first-DMA window so the HAM clock gate ramps
            # before real matmuls arrive.
            warm_in = wp.tile([128, 256], dt.bfloat16)
            nc.vector.memset(warm_in[:], 0.0)
            warm_ps = wps.tile([128, 256], dt.float32)
            for _w in range(12):
                nc.tensor.matmul(warm_ps[:, :], warm_in[:, 0:128],
                                 warm_in[:, :], start=True, stop=True)
            b_tiles = {}     # pair v -> [yt] psum tiles
            s_tiles = {}     # pair v -> sbuf fp8 tile
            n_s2 = 0
            first_s2 = True

            total_pairs = NITER
            s2_total = total_pairs * 2

            for v in range(NUST):
                if v in ust_tiles:
                    ut = ust_tiles[v]
                else:
                    ut = ustp.tile([128, 2048], dt.float8e4, tag="ust")
                    slot_dma(v, ut)
                    ust_tiles[v] = ut
                bt, off = ut, 0

                # fp8 DoubleRow: one matmul contracts both 128-z chunks.
                # lhsT [128z, 2zc, 128y@yt], rhs [128z, 2zc, 256cc]
                vw = bt[:, off + 512:off + 1024] \
                    .rearrange("p (zc y) -> p zc y", zc=WC)
                uz0 = bt[:, off + 0:off + 512] \
                    .rearrange("p (zc n) -> p zc n", zc=WC)
                uz1 = bt[:, off + 1024:off + 1536] \
                    .rearrange("p (zc n) -> p zc n", zc=WC)

                # ---- stage 1: side 0 (pair v) + side 1 (pair v-1),
                #      ordered so same-weights matmuls are adjacent ----
                s0 = v < total_pairs
                s1 = 1 <= v <= total_pairs
                if s0:
                    b_tiles[v] = [bp.tile([128, 256], dt.float32, tag=f"b{yt}",
                                          name=f"b{yt}_{v}")
                                  for yt in range(HC)]
                for yt in range(HC):
                    lt = vw[:, :, yt * 128:yt * 128 + 128]
                    if s0:
                        nc.tensor.matmul(
                            b_tiles[v][yt][:, :], lt, uz0,
                            start=True, stop=False, perf_mode=DR,
                        )
                    if s1:
                        nc.tensor.matmul(
                            b_tiles[v - 1][yt][:, :], lt, uz1,
                            start=False, stop=True, perf_mode=DR,
                        )
                if s1:
                    # copy completed B -> SBUF fp8 [128y, yc(2)*col(2)*b(128)]
                    pv = v - 1
                    st = spool.tile([128, 512], dt.float8e4, tag="s",
                                    name=f"s_{pv}")
                    for yt in range(HC):
                        dst = st[:, yt * 256:yt * 256 + 256]
                        if yt % 2 == 0:
                            nc.vector.tensor_copy(dst, b_tiles[pv][yt][:])
                        else:
                            nc.scalar.copy(dst, b_tiles[pv][yt][:])
                    s_tiles[pv] = st
                # ---- stage 2 (pair v-2), uy slices live in this iter's ut ----
                if v >= 2 and (v - 2) < total_pairs:
                    pv = v - 2
                    sv = s_tiles[pv][:].rearrange("p (yc n) -> p yc n", yc=HC)
                    for col in range(2):
                        n_s2 += 1
                        uy = bt[:, off + 1536 + col * 256:
                                off + 1536 + col * 256 + 256] \
                            .rearrange("p (yc a) -> p yc a", yc=HC)
                        nc.tensor.matmul(
                            y_ps[:, :],
                            uy,
                            sv[:, :, col * 128:col * 128 + 128],
                            start=first_s2, stop=(n_s2 == s2_total),
                            perf_mode=DR,
                        )
                        first_s2 = False
                    del s_tiles[pv]
                    del b_tiles[pv]

            y_sb = outp.tile([128, 128], dt.bfloat16)
            nc.vector.tensor_copy(y_sb[:], y_ps[:])
            # split across both HWDGE queues to halve output-DMA latency
            nc.sync.dma_start(out=y_d[:, 0:64], in_=y_sb[:, 0:64])
            nc.scalar.dma_start(out=y_d[:, 64:128], in_=y_sb[:, 64:128])

    _NC_CACHE[key] = nc
    return nc


# ----------------------------------------------------------------------------
# Entry point
# ----------------------------------------------------------------------------

def kernel(vols, sources, dests, vol_start, vol_spacing, num_steps):
    vols = np.asarray(vols, dtype=np.float32)
    sources = np.asarray(sources, dtype=np.float32)
    dests = np.asarray(dests, dtype=np.float32)
    vol_start = np.asarray(vol_start, dtype=np.float32)
    vol_spacing = np.asarray(vol_spacing, dtype=np.float32)
    num_steps = int(np.asarray(num_steps))

    g = _geometry(vols, sources, dests, vol_start, vol_spacing, num_steps)
    Ns, Nd = g['Ns'], g['Nd']
    R = Ns * Nd

    s = _build_schedule(vols, vol_start, vol_spacing, num_steps, g)
    if s is None:
        # irregular geometry: exact host fallback
        acc = _raytrace_exact(vols, vol_start, vol_spacing, num_steps, g,
                              np.ones(R, dtype=bool))
        out = np.where(g['valid'], acc, F32(0.0)).astype(F32)
        return out.reshape(Ns, Nd)

    in_maps = _build_core_inputs(vols, s)

    _install_compile_patch()
    from concourse.bass_utils import run_bass_kernel_spmd
    nc = _build_bass(s.NITER, s.NSLOT, s.H // 128, s.W_ // 128, s.H, s.W_)
    res = run_bass_kernel_spmd(nc, in_maps, core_ids=list(range(N_CORES)))
    global _LAST_RESULTS
    _LAST_RESULTS = res

    Y = np.zeros((128, 128), dtype=np.float64)
    for r in res.results:
        Y += r["y_out"].astype(np.float64)
    Y = Y.astype(F32)

    out = (g['dt'] * Y[s.a_idx, s.b_idx]).astype(F32)

    # exact recomputation for clipped / invalid rays
    fix = (~s.unclipped) & g['valid']
    if fix.any():
        out[fix] = _raytrace_exact(vols, vol_start, vol_spacing, num_steps,
                                   g, fix)
    out[~g['valid']] = 0.0
    return out.reshape(Ns, Nd)

